# revision 3
# baseline (speedup 1.0000x reference)
"""Trainium2 Bass kernel for nn_BilinearSparseRouting (FC capsule routing layer).

Math (after constant-folding the softmax-over-a-constant, which is exactly 1/32):
    cp2[b,j]   = (pose[b,j] as 4x4) @ wc[j]            # (4,4) each
    S[b]       = (1/32) * sum_j cp2[b,j]               # (4,4)
    out[b,o]   = S[b] @ wn[o]                          # (4,4), o = 0..31
    output shape (256, 1, 1, 32, 16)

Device strategy (data-parallel over batch, 32 batches per core):
  Stage 1 is a 16384-term contraction per (b, r):
      T[(b,r), c] = sum_{(j,k)} pose[b, j, 4r+k] * wc[j, k, c]
  done as 128 accumulating PE matmuls:
      lhsT = x tile (128 partitions = (j,k)-chunk, 128 cols = (b,r))   [stationary]
      rhs  = wc chunk (128 partitions, 4 cols = c)                     [moving]
  The 8 MiB/core x stream is laid out on the host so each DMA is fully
  contiguous per partition (memory-roofline streaming).
  Stage 2: transpose T/32 via PE, then one matmul against wn arranged (4, 128).
"""

import sys

for _p in ("/opt/trn_rl_repo", "/root/.axon_site/_ro/trn_rl_repo"):
    if _p not in sys.path:
        sys.path.insert(0, _p)

from contextlib import ExitStack  # noqa: E402

import numpy as np  # noqa: E402

import concourse.bacc as bacc  # noqa: E402
import concourse.mybir as mybir  # noqa: E402
import concourse.tile as tile  # noqa: E402
from concourse.bass_utils import run_bass_kernel_spmd  # noqa: E402

B = 256
N_IN = 4096
N_OUT = 32
MPD = 4
POSE_DIM = 16
N_CORES = 8
B_SH = B // N_CORES            # 32 batches per core
JK = N_IN * MPD                # 16384 contraction terms
NCHUNK = JK // 128             # 128 PE matmuls
COLS = NCHUNK * 128            # 16384 SBUF columns of x
N_GROUPS = 8                   # DMA groups for the x stream
GCOLS = COLS // N_GROUPS       # 2048 columns (1 MiB) per DMA

F32 = mybir.dt.float32

# Built once, reused across kernel() calls.
_CACHE = {}

# test.py hooks: set TRACE=True before calling kernel() to profile; the
# BassKernelResults of the last run lands in LAST_RESULT.
TRACE = False
TRACE_KWARGS = {}
LAST_RESULT = None


def _build_program():
    nc = bacc.Bacc("TRN2", target_bir_lowering=False, debug=False,
                   num_devices=N_CORES)
    x = nc.dram_tensor("x", [128, COLS], F32, kind="ExternalInput").ap()
    w = nc.dram_tensor("w", [128, NCHUNK * MPD], F32, kind="ExternalInput").ap()
    wn = nc.dram_tensor("wn", [MPD, N_OUT * MPD], F32, kind="ExternalInput").ap()
    ident = nc.dram_tensor("ident", [128, 128], F32, kind="ExternalInput").ap()
    y = nc.dram_tensor("y", [128, 128], F32, kind="ExternalOutput").ap()

    with tile.TileContext(nc) as tc, ExitStack() as ctx:
        wpool = ctx.enter_context(tc.tile_pool(name="wpool", bufs=1))
        xpool = ctx.enter_context(tc.tile_pool(name="xpool", bufs=3))
        opool = ctx.enter_context(tc.tile_pool(name="opool", bufs=1))
        ppool = ctx.enter_context(tc.tile_pool(name="ppool", bufs=1, space="PSUM"))

        w_sb = wpool.tile([128, NCHUNK * MPD], F32, tag="w")
        nc.sync.dma_start(w_sb[:], w[:])
        wn_sb = wpool.tile([MPD, N_OUT * MPD], F32, tag="wn")
        nc.sync.dma_start(wn_sb[:], wn[:])
        id_sb = wpool.tile([128, 128], F32, tag="ident")
        nc.sync.dma_start(id_sb[:], ident[:])

        # Stage 1: T[(b,r), c] accumulated over 128 chunk-matmuls.
        psum1 = ppool.tile([128, MPD], F32, tag="t")
        for g in range(N_GROUPS):
            xt = xpool.tile([128, GCOLS], F32, tag="x")
            nc.sync.dma_start(xt[:], x[:, g * GCOLS:(g + 1) * GCOLS])
            for jj in range(GCOLS // 128):
                c = g * (GCOLS // 128) + jj
                nc.tensor.matmul(
                    psum1[:],
                    lhsT=xt[:, jj * 128:(jj + 1) * 128],
                    rhs=w_sb[:, c * MPD:(c + 1) * MPD],
                    start=(c == 0),
                    stop=(c == NCHUNK - 1),
                )

        # S = T/32, then S^T via PE transpose so stage 2 can contract over k2.
        s1 = opool.tile([128, MPD], F32, tag="s1")
        nc.scalar.mul(s1[:], psum1[:], 1.0 / N_OUT)
        psum_t = ppool.tile([MPD, 128], F32, tag="st")
        nc.tensor.transpose(psum_t[:], s1[:], id_sb[:])
        s1t = opool.tile([MPD, 128], F32, tag="s1t")
        nc.scalar.copy(s1t[:], psum_t[:])

        # Stage 2: out[(b,r), (o,c)] = sum_k2 S^T[k2,(b,r)] * wn[k2,(o,c)]
        psum2 = ppool.tile([128, 128], F32, tag="out")
        nc.tensor.matmul(psum2[:], lhsT=s1t[:], rhs=wn_sb[:], start=True, stop=True)
        out_sb = opool.tile([128, 128], F32, tag="y")
        nc.vector.tensor_copy(out_sb[:], psum2[:])
        nc.sync.dma_start(y[:], out_sb[:])

    nc.compile()
    return nc


def _prep_x(current_pose: np.ndarray) -> np.ndarray:
    """(256, 4096, 16) -> (8 cores, 128 partitions, 16384 cols) SBUF image.

    Per core: row index of the contraction matrix is (j*4 + k), column is
    (b*4 + r), where pose[b, j, 4r+k] is the element.  The SBUF image packs
    chunk Jc's 128x128 tile into columns [Jc*128, (Jc+1)*128).
    """
    a = current_pose.reshape(N_CORES, B_SH, N_IN, MPD, MPD)   # m b j r k
    b = a.transpose(0, 2, 4, 1, 3)                            # m j k b r
    c = b.reshape(N_CORES, NCHUNK, 128, 128)                  # m Jc p col
    return np.ascontiguousarray(
        c.transpose(0, 2, 1, 3).reshape(N_CORES, 128, COLS))


def kernel(current_pose, w_current, w_next, h_out=1, w_out=1):
    global LAST_RESULT
    current_pose = np.asarray(current_pose, dtype=np.float32)
    w_current = np.asarray(w_current, dtype=np.float32)
    w_next = np.asarray(w_next, dtype=np.float32)

    if "nc" not in _CACHE:
        _CACHE["nc"] = _build_program()
    nc = _CACHE["nc"]

    xs = _prep_x(current_pose)
    # wc[j,k,c] flattened over (j,k) rows, packed into the (128, NCHUNK*4)
    # SBUF image the same way as x.
    wc_img = np.ascontiguousarray(
        w_current.reshape(JK, MPD).reshape(NCHUNK, 128, MPD)
        .transpose(1, 0, 2).reshape(128, NCHUNK * MPD))
    wn_img = np.ascontiguousarray(
        w_next.transpose(1, 0, 2).reshape(MPD, N_OUT * MPD))
    ident = np.eye(128, dtype=np.float32)

    in_maps = [
        {"x": xs[m], "w": wc_img, "wn": wn_img, "ident": ident}
        for m in range(N_CORES)
    ]
    res = run_bass_kernel_spmd(nc, in_maps, list(range(N_CORES)), trace=TRACE,
                               **TRACE_KWARGS)
    LAST_RESULT = res

    out = np.empty((B, 1, 1, N_OUT, POSE_DIM), dtype=np.float32)
    for m in range(N_CORES):
        ym = res.results[m]["y"]                      # (128=(b,r), 128=(o,c))
        out[m * B_SH:(m + 1) * B_SH, 0, 0] = (
            ym.reshape(B_SH, MPD, N_OUT, MPD)
            .transpose(0, 2, 1, 3).reshape(B_SH, N_OUT, POSE_DIM))
    return out


# revision 5
# speedup vs baseline: 1.4701x; 1.4701x over previous
"""Trainium2 Bass kernel for nn_BilinearSparseRouting (FC capsule routing layer).

Math (after constant-folding the softmax-over-a-constant, which is exactly 1/32):
    cp2[b,j]   = (pose[b,j] as 4x4) @ wc[j]            # (4,4) each
    S[b]       = (1/32) * sum_j cp2[b,j]               # (4,4)
    out[b,o]   = S[b] @ wn[o]                          # (4,4), o = 0..31
    output shape (256, 1, 1, 32, 16)

Device strategy (data-parallel over batch, 32 batches per core):
  Stage 1 is a 16384-term contraction per (b, r):
      T[(b,r), c] = sum_{(j,k)} pose[b, j, 4r+k] * wc[j, k, c]
  done as 128 accumulating PE matmuls:
      lhsT = x tile (128 partitions = (j,k)-chunk, 128 cols = (b,r))   [stationary]
      rhs  = wc chunk (128 partitions, 4 cols = c)                     [moving]
  The 8 MiB/core x stream is laid out on the host so each DMA is fully
  contiguous per partition (memory-roofline streaming).
  Stage 2: transpose T/32 via PE, then one matmul against wn arranged (4, 128).
"""

import sys

for _p in ("/opt/trn_rl_repo", "/root/.axon_site/_ro/trn_rl_repo"):
    if _p not in sys.path:
        sys.path.insert(0, _p)

from contextlib import ExitStack  # noqa: E402

import numpy as np  # noqa: E402

import concourse.bacc as bacc  # noqa: E402
import concourse.mybir as mybir  # noqa: E402
import concourse.tile as tile  # noqa: E402
from concourse.bass_utils import run_bass_kernel_spmd  # noqa: E402

B = 256
N_IN = 4096
N_OUT = 32
MPD = 4
POSE_DIM = 16
N_CORES = 8
B_SH = B // N_CORES            # 32 batches per core
JK = N_IN * MPD                # 16384 contraction terms
NCHUNK = JK // 128             # 128 PE matmuls
COLS = NCHUNK * 128            # 16384 SBUF columns of x
N_GROUPS = 8                   # DMA groups for the x stream
GCOLS = COLS // N_GROUPS       # 2048 columns (1 MiB) per DMA

F32 = mybir.dt.float32

# Built once, reused across kernel() calls.
_CACHE = {}

# test.py hooks: set TRACE=True before calling kernel() to profile; the
# BassKernelResults of the last run lands in LAST_RESULT.
TRACE = False
TRACE_KWARGS = {}
LAST_RESULT = None


def _build_program():
    nc = bacc.Bacc("TRN2", target_bir_lowering=False, debug=False,
                   num_devices=N_CORES)
    x = nc.dram_tensor("x", [128, COLS], F32, kind="ExternalInput").ap()
    w = nc.dram_tensor("w", [128, NCHUNK * MPD], F32, kind="ExternalInput").ap()
    wn = nc.dram_tensor("wn", [MPD, N_OUT * MPD], F32, kind="ExternalInput").ap()
    y = nc.dram_tensor("y", [128, 128], F32, kind="ExternalOutput").ap()

    with tile.TileContext(nc) as tc, ExitStack() as ctx:
        wpool = ctx.enter_context(tc.tile_pool(name="wpool", bufs=1))
        xpool = ctx.enter_context(tc.tile_pool(name="xpool", bufs=3))
        opool = ctx.enter_context(tc.tile_pool(name="opool", bufs=1))
        ppool = ctx.enter_context(tc.tile_pool(name="ppool", bufs=1, space="PSUM"))

        w_sb = wpool.tile([128, NCHUNK * MPD], F32, tag="w")
        nc.sync.dma_start(w_sb[:], w[:])
        wn_sb = wpool.tile([MPD, N_OUT * MPD], F32, tag="wn")
        nc.sync.dma_start(wn_sb[:], wn[:])

        # Stage 1: T^T[c, (b,r)] accumulated over 128 chunk-matmuls.  The
        # tiny 4-column weight chunk is the stationary operand (fp32 matmuls
        # run as 2 hi/lo passes, each reloading the stationary operand — so
        # keep it small); the x stream is the moving operand.
        psum1 = ppool.tile([MPD, 128], F32, tag="t")
        for g in range(N_GROUPS):
            xt = xpool.tile([128, GCOLS], F32, tag="x")
            nc.sync.dma_start(xt[:], x[:, g * GCOLS:(g + 1) * GCOLS])
            for jj in range(GCOLS // 128):
                c = g * (GCOLS // 128) + jj
                nc.tensor.matmul(
                    psum1[:],
                    lhsT=w_sb[:, c * MPD:(c + 1) * MPD],
                    rhs=xt[:, jj * 128:(jj + 1) * 128],
                    start=(c == 0),
                    stop=(c == NCHUNK - 1),
                )

        # S^T = T^T/32, already contraction-major for stage 2.
        s1t = opool.tile([MPD, 128], F32, tag="s1t")
        nc.scalar.mul(s1t[:], psum1[:], 1.0 / N_OUT)

        # Stage 2: out[(b,r), (o,c)] = sum_k2 S^T[k2,(b,r)] * wn[k2,(o,c)]
        psum2 = ppool.tile([128, 128], F32, tag="out")
        nc.tensor.matmul(psum2[:], lhsT=s1t[:], rhs=wn_sb[:], start=True, stop=True)
        out_sb = opool.tile([128, 128], F32, tag="y")
        nc.vector.tensor_copy(out_sb[:], psum2[:])
        nc.sync.dma_start(y[:], out_sb[:])

    nc.compile()
    return nc


def _prep_x(current_pose: np.ndarray) -> np.ndarray:
    """(256, 4096, 16) -> (8 cores, 128 partitions, 16384 cols) SBUF image.

    Per core: row index of the contraction matrix is (j*4 + k), column is
    (b*4 + r), where pose[b, j, 4r+k] is the element.  The SBUF image packs
    chunk Jc's 128x128 tile into columns [Jc*128, (Jc+1)*128).
    """
    a = current_pose.reshape(N_CORES, B_SH, N_IN, MPD, MPD)   # m b j r k
    b = a.transpose(0, 2, 4, 1, 3)                            # m j k b r
    c = b.reshape(N_CORES, NCHUNK, 128, 128)                  # m Jc p col
    return np.ascontiguousarray(
        c.transpose(0, 2, 1, 3).reshape(N_CORES, 128, COLS))


def kernel(current_pose, w_current, w_next, h_out=1, w_out=1):
    global LAST_RESULT
    current_pose = np.asarray(current_pose, dtype=np.float32)
    w_current = np.asarray(w_current, dtype=np.float32)
    w_next = np.asarray(w_next, dtype=np.float32)

    if "nc" not in _CACHE:
        _CACHE["nc"] = _build_program()
    nc = _CACHE["nc"]

    xs = _prep_x(current_pose)
    # wc[j,k,c] flattened over (j,k) rows, packed into the (128, NCHUNK*4)
    # SBUF image the same way as x.
    wc_img = np.ascontiguousarray(
        w_current.reshape(JK, MPD).reshape(NCHUNK, 128, MPD)
        .transpose(1, 0, 2).reshape(128, NCHUNK * MPD))
    wn_img = np.ascontiguousarray(
        w_next.transpose(1, 0, 2).reshape(MPD, N_OUT * MPD))

    in_maps = [
        {"x": xs[m], "w": wc_img, "wn": wn_img}
        for m in range(N_CORES)
    ]
    res = run_bass_kernel_spmd(nc, in_maps, list(range(N_CORES)), trace=TRACE,
                               **TRACE_KWARGS)
    LAST_RESULT = res

    out = np.empty((B, 1, 1, N_OUT, POSE_DIM), dtype=np.float32)
    for m in range(N_CORES):
        ym = res.results[m]["y"]                      # (128=(b,r), 128=(o,c))
        out[m * B_SH:(m + 1) * B_SH, 0, 0] = (
            ym.reshape(B_SH, MPD, N_OUT, MPD)
            .transpose(0, 2, 1, 3).reshape(B_SH, N_OUT, POSE_DIM))
    return out


# revision 6
# speedup vs baseline: 1.7749x; 1.2074x over previous
"""Trainium2 Bass kernel for nn_BilinearSparseRouting (FC capsule routing layer).

Math (after constant-folding the softmax-over-a-constant, which is exactly 1/32):
    cp2[b,j]   = (pose[b,j] as 4x4) @ wc[j]            # (4,4) each
    S[b]       = (1/32) * sum_j cp2[b,j]               # (4,4)
    out[b,o]   = S[b] @ wn[o]                          # (4,4), o = 0..31
    output shape (256, 1, 1, 32, 16)

Device strategy (data-parallel over batch, 32 batches per core):
  Stage 1 is a 16384-term contraction per (b, r):
      T[(b,r), c] = sum_{(j,k)} pose[b, j, 4r+k] * wc[j, k, c]
  done as 128 accumulating PE matmuls:
      lhsT = x tile (128 partitions = (j,k)-chunk, 128 cols = (b,r))   [stationary]
      rhs  = wc chunk (128 partitions, 4 cols = c)                     [moving]
  The 8 MiB/core x stream is laid out on the host so each DMA is fully
  contiguous per partition (memory-roofline streaming).
  Stage 2: transpose T/32 via PE, then one matmul against wn arranged (4, 128).
"""

import sys

for _p in ("/opt/trn_rl_repo", "/root/.axon_site/_ro/trn_rl_repo"):
    if _p not in sys.path:
        sys.path.insert(0, _p)

from contextlib import ExitStack  # noqa: E402

import numpy as np  # noqa: E402

import concourse.bacc as bacc  # noqa: E402
import concourse.mybir as mybir  # noqa: E402
import concourse.tile as tile  # noqa: E402
from concourse.bass_utils import run_bass_kernel_spmd  # noqa: E402

B = 256
N_IN = 4096
N_OUT = 32
MPD = 4
POSE_DIM = 16
N_CORES = 8
B_SH = B // N_CORES            # 32 batches per core
JK = N_IN * MPD                # 16384 contraction terms
NCHUNK = JK // 128             # 128 PE matmuls
COLS = NCHUNK * 128            # 16384 SBUF columns of x
N_GROUPS = 8                   # DMA groups for the x stream
GCOLS = COLS // N_GROUPS       # 2048 columns (1 MiB) per DMA

F32 = mybir.dt.float32
F32R = mybir.dt.float32r

# Built once, reused across kernel() calls.
_CACHE = {}

# test.py hooks: set TRACE=True before calling kernel() to profile; the
# BassKernelResults of the last run lands in LAST_RESULT.
TRACE = False
TRACE_KWARGS = {}
LAST_RESULT = None


def _build_program():
    nc = bacc.Bacc("TRN2", target_bir_lowering=False, debug=False,
                   num_devices=N_CORES)
    x = nc.dram_tensor("x", [128, COLS], F32R, kind="ExternalInput").ap()
    w = nc.dram_tensor("w", [128, NCHUNK * MPD], F32R, kind="ExternalInput").ap()
    wn = nc.dram_tensor("wn", [MPD, N_OUT * MPD], F32R, kind="ExternalInput").ap()
    y = nc.dram_tensor("y", [128, 128], F32, kind="ExternalOutput").ap()

    with tile.TileContext(nc) as tc, ExitStack() as ctx:
        wpool = ctx.enter_context(tc.tile_pool(name="wpool", bufs=1))
        xpool = ctx.enter_context(tc.tile_pool(name="xpool", bufs=3))
        opool = ctx.enter_context(tc.tile_pool(name="opool", bufs=1))
        ppool = ctx.enter_context(tc.tile_pool(name="ppool", bufs=1, space="PSUM"))

        w_sb = wpool.tile([128, NCHUNK * MPD], F32R, tag="w")
        nc.sync.dma_start(w_sb[:], w[:])
        wn_sb = wpool.tile([MPD, N_OUT * MPD], F32R, tag="wn")
        nc.sync.dma_start(wn_sb[:], wn[:])

        # Stage 1: T^T[c, (b,r)] accumulated over 128 chunk-matmuls.  The
        # tiny 4-column weight chunk is the stationary operand (fp32 matmuls
        # run as 2 hi/lo passes, each reloading the stationary operand — so
        # keep it small); the x stream is the moving operand.
        psum1 = ppool.tile([MPD, 128], F32, tag="t")
        for g in range(N_GROUPS):
            xt = xpool.tile([128, GCOLS], F32R, tag="x")
            nc.sync.dma_start(xt[:], x[:, g * GCOLS:(g + 1) * GCOLS])
            for jj in range(GCOLS // 128):
                c = g * (GCOLS // 128) + jj
                nc.tensor.matmul(
                    psum1[:],
                    lhsT=w_sb[:, c * MPD:(c + 1) * MPD],
                    rhs=xt[:, jj * 128:(jj + 1) * 128],
                    start=(c == 0),
                    stop=(c == NCHUNK - 1),
                )

        # S^T = T^T/32, already contraction-major for stage 2.
        s1t = opool.tile([MPD, 128], F32R, tag="s1t")
        nc.scalar.mul(s1t[:], psum1[:], 1.0 / N_OUT)

        # Stage 2: out[(b,r), (o,c)] = sum_k2 S^T[k2,(b,r)] * wn[k2,(o,c)]
        psum2 = ppool.tile([128, 128], F32, tag="out")
        nc.tensor.matmul(psum2[:], lhsT=s1t[:], rhs=wn_sb[:], start=True, stop=True)
        out_sb = opool.tile([128, 128], F32, tag="y")
        nc.vector.tensor_copy(out_sb[:], psum2[:])
        nc.sync.dma_start(y[:], out_sb[:])

    nc.compile()
    return nc


def _prep_x(current_pose: np.ndarray) -> np.ndarray:
    """(256, 4096, 16) -> (8 cores, 128 partitions, 16384 cols) SBUF image.

    Per core: row index of the contraction matrix is (j*4 + k), column is
    (b*4 + r), where pose[b, j, 4r+k] is the element.  The SBUF image packs
    chunk Jc's 128x128 tile into columns [Jc*128, (Jc+1)*128).
    """
    a = current_pose.reshape(N_CORES, B_SH, N_IN, MPD, MPD)   # m b j r k
    b = a.transpose(0, 2, 4, 1, 3)                            # m j k b r
    c = b.reshape(N_CORES, NCHUNK, 128, 128)                  # m Jc p col
    return np.ascontiguousarray(
        c.transpose(0, 2, 1, 3).reshape(N_CORES, 128, COLS))


def kernel(current_pose, w_current, w_next, h_out=1, w_out=1):
    global LAST_RESULT
    current_pose = np.asarray(current_pose, dtype=np.float32)
    w_current = np.asarray(w_current, dtype=np.float32)
    w_next = np.asarray(w_next, dtype=np.float32)

    if "nc" not in _CACHE:
        _CACHE["nc"] = _build_program()
    nc = _CACHE["nc"]

    xs = _prep_x(current_pose)
    # wc[j,k,c] flattened over (j,k) rows, packed into the (128, NCHUNK*4)
    # SBUF image the same way as x.
    wc_img = np.ascontiguousarray(
        w_current.reshape(JK, MPD).reshape(NCHUNK, 128, MPD)
        .transpose(1, 0, 2).reshape(128, NCHUNK * MPD))
    wn_img = np.ascontiguousarray(
        w_next.transpose(1, 0, 2).reshape(MPD, N_OUT * MPD))

    in_maps = [
        {"x": xs[m], "w": wc_img, "wn": wn_img}
        for m in range(N_CORES)
    ]
    res = run_bass_kernel_spmd(nc, in_maps, list(range(N_CORES)), trace=TRACE,
                               **TRACE_KWARGS)
    LAST_RESULT = res

    out = np.empty((B, 1, 1, N_OUT, POSE_DIM), dtype=np.float32)
    for m in range(N_CORES):
        ym = res.results[m]["y"]                      # (128=(b,r), 128=(o,c))
        out[m * B_SH:(m + 1) * B_SH, 0, 0] = (
            ym.reshape(B_SH, MPD, N_OUT, MPD)
            .transpose(0, 2, 1, 3).reshape(B_SH, N_OUT, POSE_DIM))
    return out
